# revision 45
# baseline (speedup 1.0000x reference)
"""Chamfer rate-distortion loss on 8 TRN2 NeuronCores — v3b.

Layout: 8 cores = 4 batches x 2 chamfer directions. Each core computes, for
its (batch, direction), the per-point nearest-neighbor squared distance of
8192 query points X against 8192 reference points Y.

Device algorithm per core:
  - PSUM holds SCALE^2 * |x-y|^2 >= 0 directly: K=13 fp16 hi/lo matmul rows
    (-2ac, -2ae, -2bc cross terms + y^2 hi/lo vs 1 + x^2 hi/lo vs 1).
  - BAND=256 (PAD=112) sorted bands over 64 blocks of 128 queries.
  - Two reduce lanes drain the single-read-ported PSUM in parallel:
      D lane (11 tiles of 4 blocks, 2 PSUM banks): DVE tensor_reduce(min)
        straight from PSUM (~1.19us/tile).
      S lane (10 tiles of 2 blocks, 1 PSUM bank): ScalarE softmin — one
        activation(Exp, scale=-16, accum_out) per block gives
        S_q = sum_j exp(-16384 d_qj); host recovers min ~= -ln(S)/16384 +
        corr, with corr calibrated per-core from two blocks computed by
        BOTH lanes (kills the softmin bias; residual ~2e-5/point).
  - Queries whose NN may fall outside their band (host Morton certificate,
    need > PAD) are gathered into 8 overflow chunks of 32; each chunk scans
    a host-chosen 512-wide rank window (chunk 4b+c at partitions 32c, bank
    b of one extra PSUM tile; a single axis-X reduce gives per-(bank,half)
    cols the host pair-mins) -- exact DVE mins for ~1.2us total.
  - Far-point padding at band edges (never a min; its exp underflows to 0).
  - Head-first DMA with issues split across the Sync and ScalarE queues;
    output DMA in 2 pieces; a dummy Exp prefetches the ACT table at t0.

Soundness: the Morton certificate proves need<=PAD queries have their NN
inside the band; hard queries are covered by their overflow window or
recomputed exactly on host; every query is checked against
est <= d_cap^2 + margin, with host recompute of violators (including all
softmin underflows, which decode to +inf).
"""

import os

import numpy as np

B, M, P = 4, 8192, 8192
SUB = 32
PAD = 104
BAND = SUB + 2 * PAD          # 240 (stored in 256-wide bank-aligned slots)
SLOT = 256
NBLK = 64                     # blocks of 128 queries
NOFCH = 8                     # overflow chunks of 32 hard queries
OFW = 512                     # overflow window width (one PSUM bank)
NOFT = NOFCH // 8             # overflow tiles (8 chunks x 512 = 2 banks)
KROWS = 13
SCALE = 32.0
S2 = SCALE * SCALE            # 1024
ACT_SCALE = -16.0             # exp(-16 * PSUM) = exp(-16384 * d)
SPRIME = -ACT_SCALE * S2      # 16384
LMBDA = 5.0

WT_W = M + NOFCH * 32                 # band stationary | OF stationary
RT_W = P + 2 * PAD                    # far | sorted Y | far
OF_W = NOFCH * OFW
TOT_W = WT_W + RT_W + OF_W
RT0 = WT_W
OF0 = WT_W + RT_W

# tile pattern: 11 'D' tiles (4 blocks, DVE exact) interleaved with 10 'S'
# tiles (2 blocks, ScalarE softmin); blocks are assigned in order, so output
# column b always holds block b (min-PSUM for D, softmin-S for S).
PATTERN = "DDSDSDSDSDSDSDSDSDSDS"
CAL_STILES = (2, 7)           # s-tile ordinals whose block 0 is also DVE'd
OFCOL = 64 + len(CAL_STILES)  # 66
NOUT = 72                     # 64 band | 2 calib | 4 OF (one OF tile) | pad

_SOFT_BLOCKS = []
_CAL_BLOCKS = []
_b = 0
_si = 0
for _t in PATTERN:
    if _t == "D":
        _b += 4
    else:
        _SOFT_BLOCKS.extend([_b, _b + 1])
        if _si in CAL_STILES:
            _CAL_BLOCKS.append(_b)
        _si += 1
        _b += 2
SOFT_BLOCKS = frozenset(_SOFT_BLOCKS)
CAL_BLOCKS = tuple(_CAL_BLOCKS)

_CACHE = {}
LAST_RESULTS = None


def _build_bass():
    import concourse.tile as tile
    from concourse import bacc, mybir

    nc = bacc.Bacc(None, target_bir_lowering=False, debug=False)
    f32 = mybir.dt.float32
    f16 = mybir.dt.float16

    wr_d = nc.dram_tensor("wr", [KROWS, TOT_W], f16, kind="ExternalInput")
    out_d = nc.dram_tensor("out", [128, NOUT], f32, kind="ExternalOutput")

    with tile.TileContext(nc) as tc:
        with (
            tc.tile_pool(name="const", bufs=1) as cpool,
            tc.tile_pool(name="outp", bufs=1) as opool,
            tc.tile_pool(name="scr", bufs=2) as spool,
            tc.tile_pool(name="psum_d", bufs=3, space="PSUM") as ppool_d,
            tc.tile_pool(name="psum_s", bufs=2, space="PSUM") as ppool_s,
        ):
            wr = cpool.tile([KROWS, TOT_W], f16)
            # head pieces issue concurrently on the sync and scalar queues
            # (rt head on sync, wt head on scalar) so the PE starts early;
            # the warm Exp (ACT table prefetch) goes right after the first
            # scalar issue so the table load overlaps the input DMA
            warm = spool.tile([128, 1], f32, tag="warm")
            nc.vector.memset(warm[:], 0.0)
            nc.sync.dma_start(wr[:, RT0:RT0 + 2048], wr_d[:, RT0:RT0 + 2048])
            nc.scalar.dma_start(wr[:, 0:1536], wr_d[:, 0:1536])
            nc.scalar.activation(warm[:], warm[:],
                                 mybir.ActivationFunctionType.Exp, scale=1.0)
            nc.sync.dma_start(wr[:, RT0 + 2048:RT0 + 4864],
                              wr_d[:, RT0 + 2048:RT0 + 4864])
            nc.scalar.dma_start(wr[:, 1536:4608], wr_d[:, 1536:4608])
            nc.sync.dma_start(wr[:, RT0 + 4864:OF0], wr_d[:, RT0 + 4864:OF0])
            nc.scalar.dma_start(wr[:, OF0:], wr_d[:, OF0:])
            nc.sync.dma_start(wr[:, 4608:RT0], wr_d[:, 4608:RT0])

            outt = opool.tile([128, NOUT], f32)

            def emit_of_tile(t):
                # 8 chunks of 32 hard queries each scan their own 512-wide
                # window: chunk 8t+4b+c sits at partitions 32c in bank b.
                # One axis-X reduce yields 4 cols (bank x half); the host
                # mins each chunk's half-pair.
                ps = ppool_d.tile([128, 2, 2, SLOT], f32, tag="ps")
                for b in range(2):
                    for c in range(4):
                        cc = 8 * t + 4 * b + c
                        nc.tensor.matmul(
                            ps[32 * c:32 * c + 32, b, :, :],
                            wr[:, M + 32 * cc:M + 32 * cc + 32],
                            wr[:, OF0 + OFW * cc:OF0 + OFW * cc + OFW],
                            start=True, stop=True,
                            tile_position=(0, 32 * c),
                        )
                nc.vector.tensor_reduce(
                    outt[:, OFCOL + 4 * t:OFCOL + 4 * t + 4], ps[:],
                    axis=mybir.AxisListType.X, op=mybir.AluOpType.min)

            blk = 0
            si = 0
            bt = 0
            half_sent = False
            for typ in PATTERN:
                if typ == "D":
                    ps = ppool_d.tile([128, 2, 2, SLOT], f32, tag="ps")
                    for j in range(4):
                        for s in range(4):
                            u = 4 * (blk + j) + s
                            nc.tensor.matmul(
                                ps[32 * s:32 * s + 32, j // 2, j % 2, 0:BAND],
                                wr[:, SUB * u:SUB * u + SUB],
                                wr[:, RT0 + SUB * u:RT0 + SUB * u + BAND],
                                start=True, stop=True,
                                tile_position=(0, 32 * s),
                            )
                    nc.vector.tensor_reduce(
                        outt[:, blk:blk + 4], ps[:, :, :, 0:BAND],
                        axis=mybir.AxisListType.X, op=mybir.AluOpType.min)
                    blk += 4
                else:
                    ps = ppool_s.tile([128, 1, 2, SLOT], f32, tag="ps")
                    for j in range(2):
                        for s in range(4):
                            u = 4 * (blk + j) + s
                            nc.tensor.matmul(
                                ps[32 * s:32 * s + 32, 0, j, 0:BAND],
                                wr[:, SUB * u:SUB * u + SUB],
                                wr[:, RT0 + SUB * u:RT0 + SUB * u + BAND],
                                start=True, stop=True,
                                tile_position=(0, 32 * s),
                            )
                    scr = spool.tile([128, 2, BAND], f32, tag="scr")
                    for j in range(2):
                        nc.scalar.activation(
                            scr[:, j, :], ps[:, 0, j, 0:BAND],
                            mybir.ActivationFunctionType.Exp,
                            scale=ACT_SCALE,
                            accum_out=outt[:, blk + j:blk + j + 1],
                        )
                    if si in CAL_STILES:
                        cj = CAL_STILES.index(si)
                        nc.vector.tensor_reduce(
                            outt[:, 64 + cj:65 + cj], ps[:, 0, 0, 0:BAND],
                            axis=mybir.AxisListType.X, op=mybir.AluOpType.min)
                    si += 1
                    blk += 2
                if blk >= 32 and not half_sent:
                    half_sent = True
                    nc.sync.dma_start(out_d[:, 0:32], outt[:, 0:32])
                bt += 1
                if bt == 12:
                    emit_of_tile(0)

            nc.sync.dma_start(out_d[:, 32:NOUT], outt[:, 32:NOUT])
    nc.compile()
    return nc


def _morton_key(pts):
    rng = pts.max(0) - pts.min(0)
    q = ((pts - pts.min(0)) / (rng + 1e-9) * 1023).astype(np.uint64)

    def spread(x):
        x = x & np.uint64(0x3FF)
        x = (x | (x << np.uint64(16))) & np.uint64(0x30000FF)
        x = (x | (x << np.uint64(8))) & np.uint64(0x300F00F)
        x = (x | (x << np.uint64(4))) & np.uint64(0x30C30C3)
        x = (x | (x << np.uint64(2))) & np.uint64(0x9249249)
        return x

    return (spread(q[:, 0]) | (spread(q[:, 1]) << np.uint64(1))
            | (spread(q[:, 2]) << np.uint64(2)))


def _prep_core(X, Y):
    """Host prep for one (batch, direction)."""
    X64 = X.astype(np.float64)
    Y64 = Y.astype(np.float64)

    # Morton d_cap (axis-independent NN upper bound from 32 candidates)
    allpts = np.concatenate([X64, Y64])
    mk = _morton_key(allpts)
    inv = np.empty(2 * M, dtype=np.int64)
    inv[np.argsort(mk, kind="stable")] = np.arange(2 * M)
    y_rank = inv[M:]
    order_y = np.argsort(y_rank, kind="stable")
    sorted_ranks = y_rank[order_y]
    idx = np.searchsorted(sorted_ranks, inv[:M])
    cand = np.clip(idx[:, None] + np.arange(-16, 16)[None, :], 0, M - 1)
    cands = order_y[cand]
    d_cap2 = ((X64[:, None, :] - Y64[cands]) ** 2).sum(-1).min(1)
    d_cap = np.sqrt(d_cap2 / 0.98)

    # choose the sort axis with the fewest hard queries
    i = np.arange(M)
    best = None
    for axis in range(3):
        xo = np.argsort(X[:, axis], kind="stable")
        yo = np.argsort(Y[:, axis], kind="stable")
        zx = X64[xo, axis]
        zy = Y64[yo, axis]
        dc = d_cap[xo]
        lo_idx = np.searchsorted(zy, zx - dc)
        hi_idx = np.searchsorted(zy, zx + dc)
        cch = i // SUB
        need = np.maximum(np.maximum(SUB * cch - lo_idx,
                                     hi_idx - (SUB * cch + SUB)), 0)
        nhard = int((need > PAD).sum())
        if best is None or nhard < best[0]:
            best = (nhard, axis, xo, yo, lo_idx, hi_idx, need)
    _, axis, xo, yo, lo_idx, hi_idx, need = best

    Xs = X64[xo]
    Ys = Y64[yo]
    X2 = (Xs ** 2).sum(1)
    Y2 = (Ys ** 2).sum(1)

    # far pad point: beyond data range along the sort axis, never a min
    zfar = np.abs(np.concatenate([Xs[:, axis], Ys[:, axis]])).max() + 2.0
    ypad = np.zeros(3)
    ypad[axis] = zfar
    Yx = np.vstack([Ys, ypad])    # index P = pad

    # fp16 hi/lo decomposition
    Xss = SCALE * Xs
    Yss = SCALE * Yx
    a = Xss.astype(np.float16)
    bb = (Xss - a.astype(np.float64)).astype(np.float16)
    c = Yss.astype(np.float16)
    e = (Yss - c.astype(np.float64)).astype(np.float16)
    w = (Yss ** 2).sum(1)
    wh = w.astype(np.float16)
    wl = (w - wh.astype(np.float64)).astype(np.float16)
    v = (Xss ** 2).sum(1)
    vh = v.astype(np.float16)
    vl = (v - vh.astype(np.float64)).astype(np.float16)
    na = (-2.0 * a.astype(np.float64)).astype(np.float16)
    nb = (-2.0 * bb.astype(np.float64)).astype(np.float16)

    # hard queries -> overflow chunks; greedy rank-run packing into 32-slot
    # chunks whose members' union [lo, hi) fits one OFW window
    hard = np.flatnonzero(need > PAD)
    of_idx = np.zeros(NOFCH * 32, dtype=np.int64)     # query per slot
    of_valid = np.zeros(NOFCH * 32, dtype=bool)
    w0s = np.zeros(NOFCH, dtype=np.int64)
    spill = []
    chunks = []
    cur, clo, chi = [], 0, 0
    for q in hard:
        lq, hq = int(lo_idx[q]), int(hi_idx[q])
        if hq - lq > OFW:
            spill.append(q)
            continue
        if cur and len(cur) < 32 and max(chi, hq) - min(clo, lq) <= OFW:
            cur.append(q)
            clo, chi = min(clo, lq), max(chi, hq)
        else:
            if cur:
                chunks.append((cur, clo, chi))
            cur, clo, chi = [q], lq, hq
    if cur:
        chunks.append((cur, clo, chi))
    for grp, clo, chi in chunks[NOFCH:]:
        spill.extend(grp)
    for ccn, (grp, clo, chi) in enumerate(chunks[:NOFCH]):
        # chi - clo <= OFW, so w0 = min(clo, P-OFW) always covers [clo, chi)
        w0 = min(clo, P - OFW)
        w0s[ccn] = w0
        nslot = len(grp)
        grp = np.asarray(grp, dtype=np.int64)
        cov = (lo_idx[grp] >= w0) & (hi_idx[grp] <= w0 + OFW)
        of_idx[32 * ccn:32 * ccn + nslot] = grp
        of_valid[32 * ccn:32 * ccn + nslot] = cov
        spill.extend(grp[~cov])
        if nslot < 32:
            of_idx[32 * ccn + nslot:32 * ccn + 32] = grp[0]

    # device input
    wr = np.zeros((KROWS, TOT_W), dtype=np.float16)
    wt = wr[:, :WT_W]
    rt = wr[:, RT0:RT0 + RT_W]
    ofr = wr[:, OF0:]

    wt[0:3, :M] = na.T
    wt[3:6, :M] = na.T
    wt[6:9, :M] = nb.T
    wt[9:11, :M] = 1.0
    wt[11, :M] = vh
    wt[12, :M] = vl
    wt[0:3, M:] = na[of_idx].T
    wt[3:6, M:] = na[of_idx].T
    wt[6:9, M:] = nb[of_idx].T
    wt[9:11, M:] = 1.0
    wt[11, M:] = vh[of_idx]
    wt[12, M:] = vl[of_idx]

    # rt: [0:PAD]=far pad, [PAD:PAD+P]=sorted Y, [PAD+P:]=far pad
    ridx = np.full(RT_W, P, dtype=np.int64)
    ridx[PAD:PAD + P] = np.arange(P)
    rt[0:3, :] = c[ridx].T
    rt[3:6, :] = e[ridx].T
    rt[6:9, :] = c[ridx].T
    rt[9, :] = wh[ridx]
    rt[10, :] = wl[ridx]
    rt[11:13, :] = 1.0

    # overflow windows (real Y columns, no pad needed)
    oidx = (w0s[:, None] + np.arange(OFW)[None, :]).reshape(-1)
    ofr[0:3, :] = c[oidx].T
    ofr[3:6, :] = e[oidx].T
    ofr[6:9, :] = c[oidx].T
    ofr[9, :] = wh[oidx]
    ofr[10, :] = wl[oidx]
    ofr[11:13, :] = 1.0

    return {"wr": wr}, {
        "Xs": Xs, "Ys": Ys, "X2": X2, "Y2": Y2,
        "hard": hard, "of_idx": of_idx, "of_valid": of_valid,
        "d_cap2": d_cap2[xo],
        "spill": np.array(sorted(set(int(s) for s in spill)), dtype=np.int64),
    }


def _post_core(out, meta):
    """Combine device output into sum over queries of min-D (float64)."""
    out = out.astype(np.float64)
    est = np.empty(M)
    softq = np.zeros(M, dtype=bool)
    for b in range(NBLK):
        vals = out[:, b]
        sl = slice(128 * b, 128 * b + 128)
        if b in SOFT_BLOCKS:
            with np.errstate(divide="ignore"):
                est[sl] = np.where(vals > 0.0,
                                   -np.log(np.maximum(vals, 1e-300)) / SPRIME,
                                   np.inf)
            softq[sl] = True
        else:
            est[sl] = vals / S2

    # softmin bias calibration from the doubly-computed blocks
    diffs = []
    for cj, b in enumerate(CAL_BLOCKS):
        exact = out[:, 64 + cj] / S2
        soft = est[128 * b:128 * b + 128]
        ok = np.isfinite(soft)
        diffs.append((exact - soft)[ok])
    dall = np.concatenate(diffs)
    corr = dall.mean() if len(dall) else 0.0
    est[softq] += corr

    # overflow results: chunk 8t+4b+c at partitions 32c, cols (bank, half)
    for t in range(NOFT):
        for b in range(2):
            vals = np.minimum(out[:, OFCOL + 4 * t + 2 * b],
                              out[:, OFCOL + 4 * t + 2 * b + 1]) / S2
            for c in range(4):
                cc = 8 * t + 4 * b + c
                for j in range(32):
                    slot = 32 * cc + j
                    if not meta["of_valid"][slot]:
                        continue
                    q = meta["of_idx"][slot]
                    v = vals[32 * c + j]
                    if v < est[q]:
                        est[q] = v

    # host-exact fixes: spill + softmin underflows + d_cap sanity violations
    fix = set(int(q) for q in meta["spill"])
    fix.update(int(q) for q in
               np.flatnonzero(~(est <= meta["d_cap2"] / 0.98 + 2e-4)))
    if fix:
        qq = np.array(sorted(fix), dtype=np.int64)
        D = (meta["Y2"][None, :] - 2.0 * (meta["Xs"][qq] @ meta["Ys"].T))
        est[qq] = D.min(1) + meta["X2"][qq]
    return est.sum()


def _install_axon_profile_hook():
    import sys
    import types
    try:
        from antenv.axon_hooks import get_axon_ntff_profile_hook  # noqa: F401
        return
    except ImportError:
        pass
    try:
        import antenv
        from trn_agent_boot.trn_boot import _ntff_profile_via_ctypes
        hook = _ntff_profile_via_ctypes("/opt/axon/libaxon_pjrt.so")
    except Exception:
        hook = None
    mod = types.ModuleType("antenv.axon_hooks")
    state = {"h": hook}
    mod.get_axon_ntff_profile_hook = lambda: state["h"]
    mod.set_axon_ntff_profile_hook = lambda h: state.__setitem__("h", h)
    sys.modules["antenv.axon_hooks"] = mod
    try:
        antenv.axon_hooks = mod
    except Exception:
        pass


def kernel(x_hat, points, likelihoods):
    from concourse.bass_utils import run_bass_kernel_spmd
    global LAST_RESULTS

    trace = bool(int(os.environ.get("CHAMFER_TRACE", "0")))
    if trace:
        _install_axon_profile_hook()

    if "nc" not in _CACHE:
        _CACHE["nc"] = _build_bass()
    nc = _CACHE["nc"]

    in_maps, metas = [], []
    for core in range(8):
        b, d = core // 2, core % 2
        X = x_hat[b] if d == 0 else points[b]
        Y = points[b] if d == 0 else x_hat[b]
        m, meta = _prep_core(np.asarray(X), np.asarray(Y))
        in_maps.append(m)
        metas.append(meta)

    res = run_bass_kernel_spmd(
        nc, in_maps, core_ids=list(range(8)), trace=trace,
    )
    LAST_RESULTS = res

    sums = [_post_core(res.results[c]["out"], metas[c]) for c in range(8)]
    cham_x = sum(sums[c] for c in range(8) if c % 2 == 0) / (B * M)
    cham_y = sum(sums[c] for c in range(8) if c % 2 == 1) / (B * P)
    rec = cham_x + cham_y

    lik = np.asarray(likelihoods, dtype=np.float64)
    bpp = np.log2(lik).sum() / (-(B * P))

    loss = bpp + LMBDA * rec
    return np.array([loss, bpp, rec], dtype=np.float32)
